# revision 12
# baseline (speedup 1.0000x reference)
"""APPNP GNN kernel for 8 Trainium2 NeuronCores (Bass/Tile).

Math: with prop(x) = segment_sum(w * x[src], dst) and zero biases,
  x1 = prop(F @ W1); x2 = prop(x1 @ W2); h = x2
  x_{k+1} = 0.9 prop(x_k) + 0.1 h   (10 steps)
prop(X) @ W == prop(X @ W), so fold Wc = W1 @ W2, G = F @ Wc, and all 12
propagations run at D=64:
  P1 = prop(G); x2 = prop(P1) = h; then 10 APPNP steps.

Device mapping (per core): nodes are in-degree-sorted into 128-row tiles;
tiles are round-robined over the 8 cores. Each dst tile has an ELL slot
block [128 pos x W cols] (W = unified max in-degree of the tile across
cores). Messages are fetched from a replicated x buffer in DRAM with
dma_gather (int16 indices; the x buffer is covered by two overlapping
32768-row windows, each edge assigned to one window with per-node
balancing). DVE fuses (msgs * scale * w) and a strided tensor_reduce per
tile writes the new shard; an ncfw AllGather rebuilds x on every core
each step.
"""
import sys
import types

import numpy as np

sys.path.insert(0, "/opt/trn_rl_repo")

import concourse.bass as bass  # noqa: E402,F401
import concourse.tile as tile  # noqa: E402
from concourse import bacc, mybir  # noqa: E402
from concourse.bass_utils import run_bass_kernel_spmd  # noqa: E402
from concourse.library_config import mlp  # noqa: E402
from concourse.tile_rust import add_dep_helper  # noqa: E402

NC = 8
TILE = 128
D_IN, D_H, D_OUT = 256, 128, 64
K_STEPS = 10
ALPHA = 0.1
CHUNK_COLS = 96          # max ELL columns buffered per chunk
GATHER_COLS = 64         # max cols (8192 idxs) per dma_gather call
REGION = 32768           # int16-addressable window rows

TRACE = False
LAST_EXEC_NS = None
EXCHANGE = True          # debug: disable collectives
N_STEPS = None           # debug: override number of propagation steps
NQUEUES = 2              # 2 SWDGE queues validated exact; 4 corrupts (see memory)
BARRIERS = False         # all-engine barriers around exchanges (not needed)
LAST_RUN_WALL_S = None


def _install_ntff_hook():
    import antenv
    if "antenv.axon_hooks" in sys.modules:
        return
    mod = types.ModuleType("antenv.axon_hooks")
    state = {"hook": None}
    mod.set_axon_ntff_profile_hook = lambda h: state.__setitem__("hook", h)
    mod.get_axon_ntff_profile_hook = lambda: state["hook"]
    sys.modules["antenv.axon_hooks"] = mod
    antenv.axon_hooks = mod
    from trn_agent_boot.trn_boot import _ntff_profile_via_ctypes
    mod.set_axon_ntff_profile_hook(
        _ntff_profile_via_ctypes("/opt/axon/libaxon_pjrt.so"))


# ---------------------------------------------------------------- host prep

def _prep(features, edge_index, edge_weight):
    n = features.shape[0]
    src = np.asarray(edge_index[0], dtype=np.int64)
    dst = np.asarray(edge_index[1], dtype=np.int64)
    w = np.asarray(edge_weight, dtype=np.float32)

    g_tiles = -(-n // TILE)
    g_tiles += (-g_tiles) % NC
    T = g_tiles // NC
    npc = T * TILE
    npad = npc * NC

    indeg = np.bincount(dst, minlength=n)
    order = np.argsort(-indeg, kind="stable")
    rank_of = np.empty(n, dtype=np.int64)
    rank_of[order] = np.arange(n)

    g = rank_of // TILE
    pos = rank_of % TILE
    core = g % NC
    t_in_core = g // NC
    phys = core * npc + pos * T + t_in_core
    # feature upload ordering: tile-major so each tile's 128 nodes are a
    # contiguous [256, 128] slice (partition p = matmul output partition)
    ftt_phys = core * npc + t_in_core * TILE + pos

    if npad <= REGION:
        b_base, two_regions = 0, False
    else:
        assert npad - REGION <= REGION
        b_base, two_regions = npad - REGION, True

    sp = phys[src]
    dp = phys[dst]
    dcore = dp // npc

    # per-core raw slot lists: slots[c][t*TILE+p] = ([(idx, w)...]A, [...]B)
    slot_a = [[[] for _ in range(T * TILE)] for _ in range(NC)]
    slot_b = [[[] for _ in range(T * TILE)] for _ in range(NC)]
    so = np.lexsort((sp, dp))
    sp_s, dp_s, w_s = sp[so], dp[so], w[so]
    bnd = np.flatnonzero(np.diff(dp_s)) + 1
    starts = np.concatenate([[0], bnd, [len(dp_s)]])
    for gi in range(len(starts) - 1):
        s0, s1 = starts[gi], starts[gi + 1]
        d = dp_s[s0]
        c = d // npc
        dl = d - c * npc
        p, t = dl // T, dl % T
        la = slot_a[c][t * TILE + p]
        lb = slot_b[c][t * TILE + p]
        if not two_regions:
            for j in range(s0, s1):
                la.append((int(sp_s[j]), float(w_s[j])))
        else:
            flex = []
            for j in range(s0, s1):
                s = int(sp_s[j])
                if s < b_base:
                    la.append((s, float(w_s[j])))
                elif s >= REGION:
                    lb.append((s - b_base, float(w_s[j])))
                else:
                    flex.append((s, float(w_s[j])))
            for s, wv in flex:
                if len(la) <= len(lb):
                    la.append((s, wv))
                else:
                    lb.append((s - b_base, wv))

    # unified widths per tile
    wa_u = np.zeros(T, dtype=np.int64)
    wb_u = np.zeros(T, dtype=np.int64)
    for c in range(NC):
        for t in range(T):
            base = t * TILE
            wa_u[t] = max(wa_u[t],
                          max((len(slot_a[c][base + p]) for p in range(TILE)),
                              default=0))
            wb_u[t] = max(wb_u[t],
                          max((len(slot_b[c][base + p]) for p in range(TILE)),
                              default=0))

    # chunks over tiles (shared structure)
    chunks = []
    t0 = 0
    while t0 < T:
        nt, cols = 0, 0
        while (t0 + nt < T
               and cols + wa_u[t0 + nt] + wb_u[t0 + nt] <= CHUNK_COLS):
            cols += wa_u[t0 + nt] + wb_u[t0 + nt]
            nt += 1
        assert nt > 0, f"tile {t0}: width {wa_u[t0]}+{wb_u[t0]} > {CHUNK_COLS}"
        chunks.append((t0, nt))
        t0 = t0 + nt

    # shared call list + per-core arrays
    calls = []           # (region, idx_off16, col_off, cn, chunk_id)
    chunk_descs = []
    idx_off = 0
    col_off = 0
    for cid, (t0, nt) in enumerate(chunks):
        cols_a = int(wa_u[t0:t0 + nt].sum())
        cols_b = int(wb_u[t0:t0 + nt].sum())
        for region, colsn in (("A", cols_a), ("B", cols_b)):
            c0 = 0
            while c0 < colsn:
                cn = min(GATHER_COLS, colsn - c0)
                calls.append((region, idx_off, col_off, cn, cid, c0))
                idx_off += cn * 8
                col_off += cn
                c0 += cn
        chunk_descs.append({
            "t0": t0, "nt": nt, "cols_a": cols_a, "cols_b": cols_b,
            "widths": [(int(wa_u[t]), int(wb_u[t]))
                       for t in range(t0, t0 + nt)]})
    tot_cols = col_off
    tot_idx16 = idx_off

    cores = []
    for c in range(NC):
        idx_arr = np.zeros((16, tot_idx16), dtype=np.int16)
        w_arr = np.zeros((TILE, tot_cols), dtype=np.float32)
        for (region, ioff, coff, cn, cid, c0) in calls:
            ch = chunk_descs[cid]
            t0, nt = ch["t0"], ch["nt"]
            widths = wa_u if region == "A" else wb_u
            slots = slot_a[c] if region == "A" else slot_b[c]
            # region columns of the chunk, flattened (t, j); call covers
            # [c0, c0+cn)
            col_list = []
            for t in range(t0, t0 + nt):
                for j in range(int(widths[t])):
                    col_list.append((t, j))
            blk_i = np.zeros((cn, TILE), dtype=np.int16)
            blk_w = np.zeros((cn, TILE), dtype=np.float32)
            for k in range(cn):
                t, j = col_list[c0 + k]
                base = t * TILE
                for p in range(TILE):
                    sl = slots[base + p]
                    if j < len(sl):
                        blk_i[k, p] = sl[j][0]
                        blk_w[k, p] = sl[j][1]
            idx_arr[:, ioff:ioff + cn * 8] = blk_i.reshape(-1, 16).T
            w_arr[:, coff:coff + cn] = blk_w.T
        cores.append({
            "idx": np.ascontiguousarray(np.tile(idx_arr, (8, 1))),
            "w": np.ascontiguousarray(w_arr)})

    meta = {"n": n, "T": T, "npc": npc, "npad": npad, "b_base": b_base,
            "phys": phys, "ftt_phys": ftt_phys, "chunks": chunk_descs,
            "calls": calls, "tot_cols": tot_cols, "tot_idx16": tot_idx16}
    return cores, meta


# ---------------------------------------------------------------- builder

def _build(meta):
    T = meta["T"]
    npc = meta["npc"]
    npad = meta["npad"]
    b_base = meta["b_base"]
    chunks = meta["chunks"]
    calls = meta["calls"]
    tot_cols = max(meta["tot_cols"], 1)
    tot_idx16 = max(meta["tot_idx16"], 16)

    nc = bacc.Bacc("TRN2", target_bir_lowering=False, debug=False,
                   num_devices=NC, num_swdge_queues=NQUEUES)
    f32 = mybir.dt.float32
    ftt = nc.dram_tensor("ftt", [D_IN, npc], f32, kind="ExternalInput")
    w1t = nc.dram_tensor("w1t", [D_H, D_IN], f32, kind="ExternalInput")
    w2 = nc.dram_tensor("w2", [D_H, D_OUT], f32, kind="ExternalInput")
    idx_in = nc.dram_tensor("idx", [TILE, tot_idx16], mybir.dt.int16,
                            kind="ExternalInput")
    wq_in = nc.dram_tensor("wq", [TILE, tot_cols], f32, kind="ExternalInput")
    out_ext = nc.dram_tensor("out", [npc, D_OUT], f32, kind="ExternalOutput")

    x_full = nc.dram_tensor("x_full", [npad, D_OUT], f32, addr_space="Shared")
    bounce = nc.dram_tensor("bounce", [npc, D_OUT], f32)

    with tile.TileContext(nc) as tc:
        with (
            tc.tile_pool(name="persist", bufs=1) as persist,
            tc.tile_pool(name="msg", bufs=2) as msgp,
            tc.tile_pool(name="ft", bufs=2) as ftp,
            tc.tile_pool(name="psum", bufs=2, space="PSUM") as psump,
        ):
            idx_t = persist.tile([TILE, tot_idx16], mybir.dt.int16)
            wq_t = persist.tile([TILE, tot_cols], f32)
            acc = persist.tile([TILE, T, D_OUT], f32)
            h01 = persist.tile([TILE, T, D_OUT], f32)
            wc0 = persist.tile([TILE, D_OUT], f32)
            wc1 = persist.tile([TILE, D_OUT], f32)
            w1t_t = persist.tile([TILE, D_IN], f32)
            w2_t = persist.tile([TILE, D_OUT], f32)

            nc.sync.dma_start(idx_t[:], idx_in[:, :])
            nc.sync.dma_start(wq_t[:], wq_in[:, :])
            nc.sync.dma_start(w1t_t[:], w1t[:, :])
            nc.sync.dma_start(w2_t[:], w2[:, :])
            nc.gpsimd.load_library(mlp)

            # Wc = W1 @ W2 -> two [128, 64] tiles
            pwc = psump.tile([TILE, D_OUT], f32, tag="pw")
            nc.tensor.matmul(pwc[:], lhsT=w1t_t[:, 0:TILE], rhs=w2_t[:],
                             start=True, stop=True)
            nc.vector.tensor_copy(wc0[:], pwc[:])
            pwc2 = psump.tile([TILE, D_OUT], f32, tag="pw")
            nc.tensor.matmul(pwc2[:], lhsT=w1t_t[:, TILE:2 * TILE], rhs=w2_t[:],
                             start=True, stop=True)
            nc.vector.tensor_copy(wc1[:], pwc2[:])

            # G = F @ Wc (this core's shard) -> acc
            for t in range(T):
                ft_t = ftp.tile([TILE, 2, TILE], f32, tag="ft")
                nc.sync.dma_start(
                    ft_t[:],
                    ftt.ap()[:, t * TILE:(t + 1) * TILE]
                    .rearrange("(h k) m -> k h m", h=2))
                pg = psump.tile([TILE, D_OUT], f32, tag="pg")
                nc.tensor.matmul(pg[:], lhsT=ft_t[:, 0, :], rhs=wc0[:],
                                 start=True, stop=False)
                nc.tensor.matmul(pg[:], lhsT=ft_t[:, 1, :], rhs=wc1[:],
                                 start=False, stop=True)
                nc.vector.tensor_copy(acc[:, t, :], pg[:])

            def exchange():
                if BARRIERS:
                    tc.strict_bb_all_engine_barrier()
                if not EXCHANGE:
                    # debug mode: single-core semantics, copy own shard into
                    # our slice of x_full
                    nc.sync.dma_start(
                        bounce.ap().rearrange("(p t) d -> p (t d)", p=TILE),
                        acc[:].rearrange("p t d -> p (t d)"))
                    nc.sync.dma_start(x_full.ap()[0:npc, :], bounce.ap())
                    return
                nc.sync.dma_start(
                    bounce.ap().rearrange("(p t) d -> p (t d)", p=TILE),
                    acc[:].rearrange("p t d -> p (t d)"))
                cc = nc.gpsimd.collective_compute(
                    "AllGather", mybir.AluOpType.bypass,
                    replica_groups=[list(range(NC))],
                    ins=[bounce.ap().opt()],
                    outs=[x_full.ap().opt()],
                )
                last_cc[0] = cc
                if BARRIERS:
                    tc.strict_bb_all_engine_barrier()

            last_cc = [None]
            exchange()

            qrot = [0]

            def step(scale, add_h):
                call_i = 0
                for ci, ch in enumerate(chunks):
                    cols_tot = ch["cols_a"] + ch["cols_b"]
                    if cols_tot == 0:
                        for i, (a, b) in enumerate(ch["widths"]):
                            pass
                        nc.vector.memset(
                            acc[:, ch["t0"]:ch["t0"] + ch["nt"], :]
                            .rearrange("p t d -> p (t d)"), 0.0)
                        continue
                    m = msgp.tile([TILE, CHUNK_COLS, D_OUT], f32, tag="m")
                    mcol = 0
                    while (call_i < len(calls) and calls[call_i][4] == ci):
                        region, ioff, coff, cn, cid, _c0 = calls[call_i]
                        base = 0 if region == "A" else b_base
                        nidx = cn * TILE
                        g = nc.gpsimd.dma_gather(
                            m[:, mcol:mcol + cn, :],
                            x_full.ap()[base:base + min(REGION, npad), :],
                            idx_t[:, ioff:ioff + cn * 8],
                            nidx, nidx, D_OUT,
                            single_packet=False,
                            queue_num=qrot[0] % NQUEUES)
                        if last_cc[0] is not None:
                            add_dep_helper(g.ins, last_cc[0].ins, sync=True,
                                           reason="gather RAW on AllGather")
                        qrot[0] += 1
                        nc.vector.scalar_tensor_tensor(
                            out=m[:, mcol:mcol + cn, :],
                            in0=m[:, mcol:mcol + cn, :],
                            scalar=float(scale),
                            in1=wq_t[:, coff:coff + cn]
                            .to_broadcast([TILE, cn, D_OUT]),
                            op0=mybir.AluOpType.mult,
                            op1=mybir.AluOpType.mult)
                        mcol += cn
                        call_i += 1
                    # per-run reductions (A and B parts of a tile are not
                    # adjacent in the buffer: columns are laid out
                    # [A(t0..), B(t0..)]; reduce A-part into acc, then add
                    # B-part on top with a second reduce into a temp? To
                    # keep one reduce per tile run, reduce A into acc and
                    # B into acc via tensor_tensor add of reduced temp.
                    for part, key in (("A", 0), ("B", 1)):
                        coff0 = 0 if part == "A" else ch["cols_a"]
                        widths = [wdt[key] for wdt in ch["widths"]]
                        coff = coff0
                        trun = ch["t0"]
                        wi = 0
                        while wi < len(widths):
                            wtot = widths[wi]
                            nt = 1
                            while (wi + nt < len(widths)
                                   and widths[wi + nt] == wtot):
                                nt += 1
                            if wtot > 0:
                                srcap = (m[:, coff:coff + nt * wtot, :]
                                         .rearrange("p (t j) f -> p t f j",
                                                    j=wtot))
                                if part == "A":
                                    nc.vector.tensor_reduce(
                                        acc[:, trun:trun + nt, :], srcap,
                                        axis=mybir.AxisListType.X,
                                        op=mybir.AluOpType.add)
                                else:
                                    tmp = msgp.tile([TILE, CHUNK_COLS, D_OUT],
                                                    f32, tag="btmp")
                                    nc.vector.tensor_reduce(
                                        tmp[:, 0:nt, :], srcap,
                                        axis=mybir.AxisListType.X,
                                        op=mybir.AluOpType.add)
                                    nc.vector.tensor_add(
                                        acc[:, trun:trun + nt, :],
                                        acc[:, trun:trun + nt, :],
                                        tmp[:, 0:nt, :])
                            elif part == "A":
                                nc.vector.memset(
                                    acc[:, trun:trun + nt, :]
                                    .rearrange("p t d -> p (t d)"), 0.0)
                            coff += nt * wtot
                            trun += nt
                            wi += nt
                if add_h:
                    nc.vector.tensor_add(
                        acc[:].rearrange("p t d -> p (t d)"),
                        acc[:].rearrange("p t d -> p (t d)"),
                        h01[:].rearrange("p t d -> p (t d)"))

            # step 1: P1 = prop(G)
            step(1.0, False)
            exchange()
            # step 2: x2 = prop(P1); h01 = 0.1 x2
            step(1.0, False)
            nc.scalar.mul(h01[:].rearrange("p t d -> p (t d)"),
                          acc[:].rearrange("p t d -> p (t d)"), ALPHA)
            exchange()
            # steps 3..12
            nk = K_STEPS if N_STEPS is None else max(0, N_STEPS - 2)
            for k in range(nk):
                step(1.0 - ALPHA, True)
                if k < nk - 1:
                    exchange()
            nc.sync.dma_start(
                out_ext.ap().rearrange("(p t) d -> p (t d)", p=TILE),
                acc[:].rearrange("p t d -> p (t d)"))

    nc.compile()
    return nc


# ---------------------------------------------------------------- entry

def kernel(features, edge_index, edge_weight, W1, b1, W2, b2):
    global LAST_EXEC_NS
    features = np.ascontiguousarray(np.asarray(features, dtype=np.float32))
    W1 = np.asarray(W1, dtype=np.float32)
    W2 = np.asarray(W2, dtype=np.float32)

    cores, meta = _prep(features, edge_index, edge_weight)
    nc = _build(meta)

    phys = meta["phys"]
    n, npc, npad = meta["n"], meta["npc"], meta["npad"]
    ftt_full = np.zeros((D_IN, npad), dtype=np.float32)
    ftt_full[:, meta["ftt_phys"]] = features.T
    in_maps = []
    for c in range(NC):
        idx = cores[c]["idx"]
        wq = cores[c]["w"]
        if meta["tot_idx16"] == 0:
            idx = np.zeros((TILE, 16), np.int16)
        if meta["tot_cols"] == 0:
            wq = np.zeros((TILE, 1), np.float32)
        in_maps.append({
            "ftt": np.ascontiguousarray(ftt_full[:, c * npc:(c + 1) * npc]),
            "w1t": np.ascontiguousarray(W1.T),
            "w2": np.ascontiguousarray(W2),
            "idx": idx,
            "wq": wq,
        })

    if TRACE:
        _install_ntff_hook()
    import time as _time
    global LAST_RUN_WALL_S
    _t0 = _time.time()
    res = run_bass_kernel_spmd(nc, in_maps, core_ids=list(range(NC)),
                               trace=TRACE)
    LAST_RUN_WALL_S = _time.time() - _t0
    LAST_EXEC_NS = res.exec_time_ns

    out_full = np.concatenate([res.results[c]["out"] for c in range(NC)], 0)
    out = out_full[phys[np.arange(n)]]
    return (np.ascontiguousarray(out, dtype=np.float32), 10)


# revision 14
# speedup vs baseline: 1.0547x; 1.0547x over previous
"""APPNP GNN kernel for 8 Trainium2 NeuronCores (Bass/Tile).

Math: with prop(x) = segment_sum(w * x[src], dst) and zero biases,
  x1 = prop(F @ W1); x2 = prop(x1 @ W2); h = x2
  x_{k+1} = 0.9 prop(x_k) + 0.1 h   (10 steps)
prop(X) @ W == prop(X @ W), so fold Wc = W1 @ W2, G = F @ Wc, and all 12
propagations run at D=64:
  P1 = prop(G); x2 = prop(P1) = h; then 10 APPNP steps.

Device mapping (per core): nodes are in-degree-sorted into 128-row tiles;
tiles are round-robined over the 8 cores. Each dst tile has an ELL slot
block [128 pos x W cols] (W = unified max in-degree of the tile across
cores). Messages are fetched from a replicated x buffer in DRAM with
dma_gather (int16 indices; the x buffer is covered by two overlapping
32768-row windows, each edge assigned to one window with per-node
balancing). DVE fuses (msgs * scale * w) and a strided tensor_reduce per
tile writes the new shard; an ncfw AllGather rebuilds x on every core
each step.
"""
import sys
import types

import numpy as np

sys.path.insert(0, "/opt/trn_rl_repo")

import concourse.bass as bass  # noqa: E402,F401
import concourse.tile as tile  # noqa: E402
from concourse import bacc, mybir  # noqa: E402
from concourse.bass_utils import run_bass_kernel_spmd  # noqa: E402
from concourse.library_config import mlp  # noqa: E402
from concourse.tile_rust import add_dep_helper  # noqa: E402

NC = 8
TILE = 128
D_IN, D_H, D_OUT = 256, 128, 64
K_STEPS = 10
ALPHA = 0.1
CHUNK_COLS = 96          # max ELL columns buffered per chunk
GATHER_COLS = 64         # max cols (8192 idxs) per dma_gather call
REGION = 32768           # int16-addressable window rows

TRACE = False
LAST_EXEC_NS = None
EXCHANGE = True          # debug: disable collectives
N_STEPS = None           # debug: override number of propagation steps
NQUEUES = 2              # 2 SWDGE queues validated exact; 4 corrupts (see memory)
BARRIERS = False         # all-engine barriers around exchanges (not needed)
LAST_RUN_WALL_S = None


def _install_ntff_hook():
    import antenv
    if "antenv.axon_hooks" in sys.modules:
        return
    mod = types.ModuleType("antenv.axon_hooks")
    state = {"hook": None}
    mod.set_axon_ntff_profile_hook = lambda h: state.__setitem__("hook", h)
    mod.get_axon_ntff_profile_hook = lambda: state["hook"]
    sys.modules["antenv.axon_hooks"] = mod
    antenv.axon_hooks = mod
    from trn_agent_boot.trn_boot import _ntff_profile_via_ctypes
    mod.set_axon_ntff_profile_hook(
        _ntff_profile_via_ctypes("/opt/axon/libaxon_pjrt.so"))


# ---------------------------------------------------------------- host prep

def _prep(features, edge_index, edge_weight):
    n = features.shape[0]
    src = np.asarray(edge_index[0], dtype=np.int64)
    dst = np.asarray(edge_index[1], dtype=np.int64)
    w = np.asarray(edge_weight, dtype=np.float32)

    g_tiles = -(-n // TILE)
    g_tiles += (-g_tiles) % NC
    T = g_tiles // NC
    npc = T * TILE
    npad = npc * NC

    indeg = np.bincount(dst, minlength=n)
    order = np.argsort(-indeg, kind="stable")
    rank_of = np.empty(n, dtype=np.int64)
    rank_of[order] = np.arange(n)

    g = rank_of // TILE
    pos = rank_of % TILE
    core = g % NC
    t_in_core = g // NC
    phys = core * npc + pos * T + t_in_core
    # feature upload ordering: tile-major so each tile's 128 nodes are a
    # contiguous [256, 128] slice (partition p = matmul output partition)
    ftt_phys = core * npc + t_in_core * TILE + pos

    if npad <= REGION:
        b_base, two_regions = 0, False
    else:
        assert npad - REGION <= REGION
        b_base, two_regions = npad - REGION, True

    sp = phys[src]
    dp = phys[dst]
    dcore = dp // npc

    # per-core raw slot lists: slots[c][t*TILE+p] = ([(idx, w)...]A, [...]B)
    slot_a = [[[] for _ in range(T * TILE)] for _ in range(NC)]
    slot_b = [[[] for _ in range(T * TILE)] for _ in range(NC)]
    so = np.lexsort((sp, dp))
    sp_s, dp_s, w_s = sp[so], dp[so], w[so]
    bnd = np.flatnonzero(np.diff(dp_s)) + 1
    starts = np.concatenate([[0], bnd, [len(dp_s)]])
    for gi in range(len(starts) - 1):
        s0, s1 = starts[gi], starts[gi + 1]
        d = dp_s[s0]
        c = d // npc
        dl = d - c * npc
        p, t = dl // T, dl % T
        la = slot_a[c][t * TILE + p]
        lb = slot_b[c][t * TILE + p]
        if not two_regions:
            for j in range(s0, s1):
                la.append((int(sp_s[j]), float(w_s[j])))
        else:
            flex = []
            for j in range(s0, s1):
                s = int(sp_s[j])
                if s < b_base:
                    la.append((s, float(w_s[j])))
                elif s >= REGION:
                    lb.append((s - b_base, float(w_s[j])))
                else:
                    flex.append((s, float(w_s[j])))
            for s, wv in flex:
                if len(la) <= len(lb):
                    la.append((s, wv))
                else:
                    lb.append((s - b_base, wv))

    # unified widths per tile
    wa_u = np.zeros(T, dtype=np.int64)
    wb_u = np.zeros(T, dtype=np.int64)
    for c in range(NC):
        for t in range(T):
            base = t * TILE
            wa_u[t] = max(wa_u[t],
                          max((len(slot_a[c][base + p]) for p in range(TILE)),
                              default=0))
            wb_u[t] = max(wb_u[t],
                          max((len(slot_b[c][base + p]) for p in range(TILE)),
                              default=0))

    # chunks over tiles (shared structure)
    chunks = []
    t0 = 0
    while t0 < T:
        nt, cols = 0, 0
        while (t0 + nt < T
               and cols + wa_u[t0 + nt] + wb_u[t0 + nt] <= CHUNK_COLS):
            cols += wa_u[t0 + nt] + wb_u[t0 + nt]
            nt += 1
        assert nt > 0, f"tile {t0}: width {wa_u[t0]}+{wb_u[t0]} > {CHUNK_COLS}"
        chunks.append((t0, nt))
        t0 = t0 + nt

    # shared call list + per-core arrays
    calls = []           # (region, idx_off16, col_off, cn, chunk_id)
    chunk_descs = []
    idx_off = 0
    col_off = 0
    for cid, (t0, nt) in enumerate(chunks):
        cols_a = int(wa_u[t0:t0 + nt].sum())
        cols_b = int(wb_u[t0:t0 + nt].sum())
        for region, colsn in (("A", cols_a), ("B", cols_b)):
            c0 = 0
            while c0 < colsn:
                cn = min(GATHER_COLS, colsn - c0)
                calls.append((region, idx_off, col_off, cn, cid, c0))
                idx_off += cn * 8
                col_off += cn
                c0 += cn
        chunk_descs.append({
            "t0": t0, "nt": nt, "cols_a": cols_a, "cols_b": cols_b,
            "widths": [(int(wa_u[t]), int(wb_u[t]))
                       for t in range(t0, t0 + nt)]})
    tot_cols = col_off
    tot_idx16 = idx_off

    cores = []
    for c in range(NC):
        idx_arr = np.zeros((16, tot_idx16), dtype=np.int16)
        w_arr = np.zeros((TILE, tot_cols), dtype=np.float32)
        for (region, ioff, coff, cn, cid, c0) in calls:
            ch = chunk_descs[cid]
            t0, nt = ch["t0"], ch["nt"]
            widths = wa_u if region == "A" else wb_u
            slots = slot_a[c] if region == "A" else slot_b[c]
            # region columns of the chunk, flattened (t, j); call covers
            # [c0, c0+cn)
            col_list = []
            for t in range(t0, t0 + nt):
                for j in range(int(widths[t])):
                    col_list.append((t, j))
            blk_i = np.zeros((cn, TILE), dtype=np.int16)
            blk_w = np.zeros((cn, TILE), dtype=np.float32)
            for k in range(cn):
                t, j = col_list[c0 + k]
                base = t * TILE
                for p in range(TILE):
                    sl = slots[base + p]
                    if j < len(sl):
                        blk_i[k, p] = sl[j][0]
                        blk_w[k, p] = sl[j][1]
            idx_arr[:, ioff:ioff + cn * 8] = blk_i.reshape(-1, 16).T
            w_arr[:, coff:coff + cn] = blk_w.T
        cores.append({
            "idx": np.ascontiguousarray(np.tile(idx_arr, (8, 1))),
            "w": np.ascontiguousarray(w_arr)})

    meta = {"n": n, "T": T, "npc": npc, "npad": npad, "b_base": b_base,
            "phys": phys, "ftt_phys": ftt_phys, "chunks": chunk_descs,
            "calls": calls, "tot_cols": tot_cols, "tot_idx16": tot_idx16}
    return cores, meta


# ---------------------------------------------------------------- builder

def _build(meta):
    T = meta["T"]
    npc = meta["npc"]
    npad = meta["npad"]
    b_base = meta["b_base"]
    chunks = meta["chunks"]
    calls = meta["calls"]
    tot_cols = max(meta["tot_cols"], 1)
    tot_idx16 = max(meta["tot_idx16"], 16)

    nc = bacc.Bacc("TRN2", target_bir_lowering=False, debug=False,
                   num_devices=NC, num_swdge_queues=NQUEUES)
    f32 = mybir.dt.float32
    ftt = nc.dram_tensor("ftt", [D_IN, npc], f32, kind="ExternalInput")
    w1t = nc.dram_tensor("w1t", [D_H, D_IN], f32, kind="ExternalInput")
    w2 = nc.dram_tensor("w2", [D_H, D_OUT], f32, kind="ExternalInput")
    idx_in = nc.dram_tensor("idx", [TILE, tot_idx16], mybir.dt.int16,
                            kind="ExternalInput")
    wq_in = nc.dram_tensor("wq", [TILE, tot_cols], f32, kind="ExternalInput")
    out_ext = nc.dram_tensor("out", [npc, D_OUT], f32, kind="ExternalOutput")

    x_full = nc.dram_tensor("x_full", [npad, D_OUT], f32, addr_space="Shared")
    bounce = nc.dram_tensor("bounce", [npc, D_OUT], f32)

    with tile.TileContext(nc) as tc:
        with (
            tc.tile_pool(name="persist", bufs=1) as persist,
            tc.tile_pool(name="msg", bufs=2) as msgp,
            tc.tile_pool(name="ft", bufs=2) as ftp,
            tc.tile_pool(name="psum", bufs=2, space="PSUM") as psump,
        ):
            idx_t = persist.tile([TILE, tot_idx16], mybir.dt.int16)
            wq_t = persist.tile([TILE, tot_cols], f32)
            acc = persist.tile([TILE, T, D_OUT], f32)
            h01 = persist.tile([TILE, T, D_OUT], f32)
            wc0 = persist.tile([TILE, D_OUT], f32)
            wc1 = persist.tile([TILE, D_OUT], f32)
            w1t_t = persist.tile([TILE, D_IN], f32)
            w2_t = persist.tile([TILE, D_OUT], f32)

            nc.sync.dma_start(idx_t[:], idx_in[:, :])
            nc.sync.dma_start(wq_t[:], wq_in[:, :])
            nc.sync.dma_start(w1t_t[:], w1t[:, :])
            nc.sync.dma_start(w2_t[:], w2[:, :])
            nc.gpsimd.load_library(mlp)

            # Wc = W1 @ W2 -> two [128, 64] tiles
            pwc = psump.tile([TILE, D_OUT], f32, tag="pw")
            nc.tensor.matmul(pwc[:], lhsT=w1t_t[:, 0:TILE], rhs=w2_t[:],
                             start=True, stop=True)
            nc.vector.tensor_copy(wc0[:], pwc[:])
            pwc2 = psump.tile([TILE, D_OUT], f32, tag="pw")
            nc.tensor.matmul(pwc2[:], lhsT=w1t_t[:, TILE:2 * TILE], rhs=w2_t[:],
                             start=True, stop=True)
            nc.vector.tensor_copy(wc1[:], pwc2[:])

            # G = F @ Wc (this core's shard) -> acc
            for t in range(T):
                ft_t = ftp.tile([TILE, 2, TILE], f32, tag="ft")
                nc.sync.dma_start(
                    ft_t[:],
                    ftt.ap()[:, t * TILE:(t + 1) * TILE]
                    .rearrange("(h k) m -> k h m", h=2))
                pg = psump.tile([TILE, D_OUT], f32, tag="pg")
                nc.tensor.matmul(pg[:], lhsT=ft_t[:, 0, :], rhs=wc0[:],
                                 start=True, stop=False)
                nc.tensor.matmul(pg[:], lhsT=ft_t[:, 1, :], rhs=wc1[:],
                                 start=False, stop=True)
                nc.vector.tensor_copy(acc[:, t, :], pg[:])

            def exchange():
                if BARRIERS:
                    tc.strict_bb_all_engine_barrier()
                if not EXCHANGE:
                    # debug mode: single-core semantics, copy own shard into
                    # our slice of x_full
                    nc.sync.dma_start(
                        bounce.ap().rearrange("(p t) d -> p (t d)", p=TILE),
                        acc[:].rearrange("p t d -> p (t d)"))
                    nc.sync.dma_start(x_full.ap()[0:npc, :], bounce.ap())
                    return
                nc.sync.dma_start(
                    bounce.ap().rearrange("(p t) d -> p (t d)", p=TILE),
                    acc[:].rearrange("p t d -> p (t d)"))
                cc = nc.gpsimd.collective_compute(
                    "AllGather", mybir.AluOpType.bypass,
                    replica_groups=[list(range(NC))],
                    ins=[bounce.ap().opt()],
                    outs=[x_full.ap().opt()],
                )
                last_cc[0] = cc
                if BARRIERS:
                    tc.strict_bb_all_engine_barrier()

            last_cc = [None]
            exchange()

            qrot = [0]

            def step(scale, add_h):
                call_i = 0
                for ci, ch in enumerate(chunks):
                    cols_tot = ch["cols_a"] + ch["cols_b"]
                    if cols_tot == 0:
                        for i, (a, b) in enumerate(ch["widths"]):
                            pass
                        nc.vector.memset(
                            acc[:, ch["t0"]:ch["t0"] + ch["nt"], :]
                            .rearrange("p t d -> p (t d)"), 0.0)
                        continue
                    m = msgp.tile([TILE, CHUNK_COLS, D_OUT], f32, tag="m")
                    mcol = 0
                    while (call_i < len(calls) and calls[call_i][4] == ci):
                        region, ioff, coff, cn, cid, _c0 = calls[call_i]
                        base = 0 if region == "A" else b_base
                        nidx = cn * TILE
                        g = nc.gpsimd.dma_gather(
                            m[:, mcol:mcol + cn, :],
                            x_full.ap()[base:base + min(REGION, npad), :],
                            idx_t[:, ioff:ioff + cn * 8],
                            nidx, nidx, D_OUT,
                            single_packet=False,
                            queue_num=qrot[0] % NQUEUES)
                        if last_cc[0] is not None:
                            add_dep_helper(g.ins, last_cc[0].ins, sync=True,
                                           reason="gather RAW on AllGather")
                        qrot[0] += 1
                        nc.vector.scalar_tensor_tensor(
                            out=m[:, mcol:mcol + cn, :],
                            in0=m[:, mcol:mcol + cn, :],
                            scalar=float(scale),
                            in1=wq_t[:, coff:coff + cn]
                            .to_broadcast([TILE, cn, D_OUT]),
                            op0=mybir.AluOpType.mult,
                            op1=mybir.AluOpType.mult)
                        mcol += cn
                        call_i += 1
                    # per-run reductions (A and B parts of a tile are not
                    # adjacent in the buffer: columns are laid out
                    # [A(t0..), B(t0..)]; reduce A-part into acc, then add
                    # B-part on top with a second reduce into a temp? To
                    # keep one reduce per tile run, reduce A into acc and
                    # B into acc via tensor_tensor add of reduced temp.
                    for part, key in (("A", 0), ("B", 1)):
                        coff0 = 0 if part == "A" else ch["cols_a"]
                        widths = [wdt[key] for wdt in ch["widths"]]
                        coff = coff0
                        trun = ch["t0"]
                        wi = 0
                        while wi < len(widths):
                            wtot = widths[wi]
                            nt = 1
                            while (wi + nt < len(widths)
                                   and widths[wi + nt] == wtot):
                                nt += 1
                            if wtot > 0:
                                srcap = (m[:, coff:coff + nt * wtot, :]
                                         .rearrange("p (t j) f -> p t f j",
                                                    j=wtot))
                                if part == "A":
                                    nc.vector.tensor_reduce(
                                        acc[:, trun:trun + nt, :], srcap,
                                        axis=mybir.AxisListType.X,
                                        op=mybir.AluOpType.add)
                                else:
                                    tmp = msgp.tile([TILE, CHUNK_COLS, D_OUT],
                                                    f32, tag="btmp")
                                    nc.vector.tensor_reduce(
                                        tmp[:, 0:nt, :], srcap,
                                        axis=mybir.AxisListType.X,
                                        op=mybir.AluOpType.add)
                                    nc.vector.tensor_add(
                                        acc[:, trun:trun + nt, :],
                                        acc[:, trun:trun + nt, :],
                                        tmp[:, 0:nt, :])
                            elif part == "A":
                                nc.vector.memset(
                                    acc[:, trun:trun + nt, :]
                                    .rearrange("p t d -> p (t d)"), 0.0)
                            coff += nt * wtot
                            trun += nt
                            wi += nt
                if add_h:
                    nc.vector.tensor_add(
                        acc[:].rearrange("p t d -> p (t d)"),
                        acc[:].rearrange("p t d -> p (t d)"),
                        h01[:].rearrange("p t d -> p (t d)"))

            # step 1: P1 = prop(G)
            step(1.0, False)
            exchange()
            # step 2: x2 = prop(P1); h01 = 0.1 x2
            step(1.0, False)
            nc.scalar.mul(h01[:].rearrange("p t d -> p (t d)"),
                          acc[:].rearrange("p t d -> p (t d)"), ALPHA)
            exchange()
            # steps 3..12
            nk = K_STEPS if N_STEPS is None else max(0, N_STEPS - 2)
            for k in range(nk):
                step(1.0 - ALPHA, True)
                if k < nk - 1:
                    exchange()
            nc.sync.dma_start(
                out_ext.ap().rearrange("(p t) d -> p (t d)", p=TILE),
                acc[:].rearrange("p t d -> p (t d)"))

    nc.compile()
    return nc


# ---------------------------------------------------------------- entry

def kernel(features, edge_index, edge_weight, W1, b1, W2, b2):
    global LAST_EXEC_NS
    features = np.ascontiguousarray(np.asarray(features, dtype=np.float32))
    W1 = np.asarray(W1, dtype=np.float32)
    W2 = np.asarray(W2, dtype=np.float32)

    cores, meta = _prep(features, edge_index, edge_weight)
    nc = _build(meta)

    phys = meta["phys"]
    n, npc, npad = meta["n"], meta["npc"], meta["npad"]
    ftt_full = np.zeros((D_IN, npad), dtype=np.float32)
    ftt_full[:, meta["ftt_phys"]] = features.T
    in_maps = []
    for c in range(NC):
        idx = cores[c]["idx"]
        wq = cores[c]["w"]
        if meta["tot_idx16"] == 0:
            idx = np.zeros((TILE, 16), np.int16)
        if meta["tot_cols"] == 0:
            wq = np.zeros((TILE, 1), np.float32)
        in_maps.append({
            "ftt": np.ascontiguousarray(ftt_full[:, c * npc:(c + 1) * npc]),
            "w1t": np.ascontiguousarray(W1.T),
            "w2": np.ascontiguousarray(W2),
            "idx": idx,
            "wq": wq,
        })

    if TRACE:
        _install_ntff_hook()
    import time as _time
    global LAST_RUN_WALL_S
    _t0 = _time.time()
    res = run_bass_kernel_spmd(nc, in_maps, core_ids=list(range(NC)),
                               trace=TRACE)
    LAST_RUN_WALL_S = _time.time() - _t0
    LAST_EXEC_NS = res.exec_time_ns

    out_full = np.concatenate([res.results[c]["out"] for c in range(NC)], 0)
    out = out_full[phys[np.arange(n)]]
    return (np.ascontiguousarray(out, dtype=np.float32), 10)


# revision 16
# speedup vs baseline: 1.0604x; 1.0055x over previous
"""APPNP GNN kernel for 8 Trainium2 NeuronCores (Bass/Tile).

Math: with prop(x) = segment_sum(w * x[src], dst) and zero biases,
  x1 = prop(F @ W1); x2 = prop(x1 @ W2); h = x2
  x_{k+1} = 0.9 prop(x_k) + 0.1 h   (10 steps)
prop(X) @ W == prop(X @ W), so fold Wc = W1 @ W2, G = F @ Wc, and all 12
propagations run at D=64:
  P1 = prop(G); x2 = prop(P1) = h; then 10 APPNP steps.

Device mapping (per core): nodes are in-degree-sorted into 128-row tiles;
tiles are round-robined over the 8 cores. Each dst tile has an ELL slot
block [128 pos x W cols] (W = unified max in-degree of the tile across
cores). Messages are fetched from a replicated x buffer in DRAM with
dma_gather (int16 indices; the x buffer is covered by two overlapping
32768-row windows, each edge assigned to one window with per-node
balancing). DVE fuses (msgs * scale * w) and a strided tensor_reduce per
tile writes the new shard; an ncfw AllGather rebuilds x on every core
each step.
"""
import sys
import types

import numpy as np

sys.path.insert(0, "/opt/trn_rl_repo")

import concourse.bass as bass  # noqa: E402,F401
import concourse.tile as tile  # noqa: E402
from concourse import bacc, mybir  # noqa: E402
from concourse.bass_utils import run_bass_kernel_spmd  # noqa: E402
from concourse.library_config import mlp  # noqa: E402
from concourse.tile_rust import add_dep_helper  # noqa: E402

NC = 8
TILE = 128
D_IN, D_H, D_OUT = 256, 128, 64
K_STEPS = 10
ALPHA = 0.1
CHUNK_COLS = 96          # max ELL columns buffered per chunk
GATHER_COLS = 64         # max cols (8192 idxs) per dma_gather call
REGION = 32768           # int16-addressable window rows

TRACE = False
LAST_EXEC_NS = None
EXCHANGE = True          # debug: disable collectives
N_STEPS = None           # debug: override number of propagation steps
NQUEUES = 2              # 2 SWDGE queues validated exact; 4 corrupts (see memory)
BARRIERS = False         # all-engine barriers around exchanges (not needed)
LAST_RUN_WALL_S = None


def _install_ntff_hook():
    import antenv
    if "antenv.axon_hooks" in sys.modules:
        return
    mod = types.ModuleType("antenv.axon_hooks")
    state = {"hook": None}
    mod.set_axon_ntff_profile_hook = lambda h: state.__setitem__("hook", h)
    mod.get_axon_ntff_profile_hook = lambda: state["hook"]
    sys.modules["antenv.axon_hooks"] = mod
    antenv.axon_hooks = mod
    from trn_agent_boot.trn_boot import _ntff_profile_via_ctypes
    mod.set_axon_ntff_profile_hook(
        _ntff_profile_via_ctypes("/opt/axon/libaxon_pjrt.so"))


# ---------------------------------------------------------------- host prep

def _prep(features, edge_index, edge_weight):
    n = features.shape[0]
    src = np.asarray(edge_index[0], dtype=np.int64)
    dst = np.asarray(edge_index[1], dtype=np.int64)
    w = np.asarray(edge_weight, dtype=np.float32)

    g_tiles = -(-n // TILE)
    g_tiles += (-g_tiles) % NC
    T = g_tiles // NC
    npc = T * TILE
    npad = npc * NC

    indeg = np.bincount(dst, minlength=n)
    order = np.argsort(-indeg, kind="stable")
    rank_of = np.empty(n, dtype=np.int64)
    rank_of[order] = np.arange(n)

    g = rank_of // TILE
    pos = rank_of % TILE
    core = g % NC
    t_in_core = g // NC
    phys = core * npc + pos * T + t_in_core
    # feature upload ordering: tile-major so each tile's 128 nodes are a
    # contiguous [256, 128] slice (partition p = matmul output partition)
    ftt_phys = core * npc + t_in_core * TILE + pos

    if npad <= REGION:
        b_base, two_regions = 0, False
    else:
        assert npad - REGION <= REGION
        b_base, two_regions = npad - REGION, True

    sp = phys[src]
    dp = phys[dst]
    dcore = dp // npc

    # per-core raw slot lists: slots[c][t*TILE+p] = ([(idx, w)...]A, [...]B)
    slot_a = [[[] for _ in range(T * TILE)] for _ in range(NC)]
    slot_b = [[[] for _ in range(T * TILE)] for _ in range(NC)]
    so = np.lexsort((sp, dp))
    sp_s, dp_s, w_s = sp[so], dp[so], w[so]
    bnd = np.flatnonzero(np.diff(dp_s)) + 1
    starts = np.concatenate([[0], bnd, [len(dp_s)]])
    for gi in range(len(starts) - 1):
        s0, s1 = starts[gi], starts[gi + 1]
        d = dp_s[s0]
        c = d // npc
        dl = d - c * npc
        p, t = dl // T, dl % T
        la = slot_a[c][t * TILE + p]
        lb = slot_b[c][t * TILE + p]
        if not two_regions:
            for j in range(s0, s1):
                la.append((int(sp_s[j]), float(w_s[j])))
        else:
            flex = []
            for j in range(s0, s1):
                s = int(sp_s[j])
                if s < b_base:
                    la.append((s, float(w_s[j])))
                elif s >= REGION:
                    lb.append((s - b_base, float(w_s[j])))
                else:
                    flex.append((s, float(w_s[j])))
            for s, wv in flex:
                if len(la) <= len(lb):
                    la.append((s, wv))
                else:
                    lb.append((s - b_base, wv))

    # unified widths per tile
    wa_u = np.zeros(T, dtype=np.int64)
    wb_u = np.zeros(T, dtype=np.int64)
    for c in range(NC):
        for t in range(T):
            base = t * TILE
            wa_u[t] = max(wa_u[t],
                          max((len(slot_a[c][base + p]) for p in range(TILE)),
                              default=0))
            wb_u[t] = max(wb_u[t],
                          max((len(slot_b[c][base + p]) for p in range(TILE)),
                              default=0))

    # chunks over tiles (shared structure)
    chunks = []
    t0 = 0
    while t0 < T:
        nt, cols = 0, 0
        while (t0 + nt < T
               and cols + wa_u[t0 + nt] + wb_u[t0 + nt] <= CHUNK_COLS):
            cols += wa_u[t0 + nt] + wb_u[t0 + nt]
            nt += 1
        assert nt > 0, f"tile {t0}: width {wa_u[t0]}+{wb_u[t0]} > {CHUNK_COLS}"
        chunks.append((t0, nt))
        t0 = t0 + nt

    # shared call list + per-core arrays
    calls = []           # (region, idx_off16, col_off, cn, chunk_id)
    chunk_descs = []
    idx_off = 0
    col_off = 0
    for cid, (t0, nt) in enumerate(chunks):
        cols_a = int(wa_u[t0:t0 + nt].sum())
        cols_b = int(wb_u[t0:t0 + nt].sum())
        for region, colsn in (("A", cols_a), ("B", cols_b)):
            c0 = 0
            while c0 < colsn:
                cn = min(GATHER_COLS, colsn - c0)
                calls.append((region, idx_off, col_off, cn, cid, c0))
                idx_off += cn * 8
                col_off += cn
                c0 += cn
        chunk_descs.append({
            "t0": t0, "nt": nt, "cols_a": cols_a, "cols_b": cols_b,
            "widths": [(int(wa_u[t]), int(wb_u[t]))
                       for t in range(t0, t0 + nt)]})
    tot_cols = col_off
    tot_idx16 = idx_off

    cores = []
    for c in range(NC):
        idx_arr = np.zeros((16, tot_idx16), dtype=np.int16)
        w_arr = np.zeros((TILE, tot_cols), dtype=np.float32)
        for (region, ioff, coff, cn, cid, c0) in calls:
            ch = chunk_descs[cid]
            t0, nt = ch["t0"], ch["nt"]
            widths = wa_u if region == "A" else wb_u
            slots = slot_a[c] if region == "A" else slot_b[c]
            # region columns of the chunk, flattened (t, j); call covers
            # [c0, c0+cn)
            col_list = []
            for t in range(t0, t0 + nt):
                for j in range(int(widths[t])):
                    col_list.append((t, j))
            blk_i = np.zeros((cn, TILE), dtype=np.int16)
            blk_w = np.zeros((cn, TILE), dtype=np.float32)
            for k in range(cn):
                t, j = col_list[c0 + k]
                base = t * TILE
                for p in range(TILE):
                    sl = slots[base + p]
                    if j < len(sl):
                        blk_i[k, p] = sl[j][0]
                        blk_w[k, p] = sl[j][1]
            idx_arr[:, ioff:ioff + cn * 8] = blk_i.reshape(-1, 16).T
            w_arr[:, coff:coff + cn] = blk_w.T
        cores.append({
            "idx": np.ascontiguousarray(np.tile(idx_arr, (8, 1))),
            "w": np.ascontiguousarray(w_arr)})

    meta = {"n": n, "T": T, "npc": npc, "npad": npad, "b_base": b_base,
            "phys": phys, "ftt_phys": ftt_phys, "chunks": chunk_descs,
            "calls": calls, "tot_cols": tot_cols, "tot_idx16": tot_idx16}
    return cores, meta


# ---------------------------------------------------------------- builder

def _build(meta):
    T = meta["T"]
    npc = meta["npc"]
    npad = meta["npad"]
    b_base = meta["b_base"]
    chunks = meta["chunks"]
    calls = meta["calls"]
    max_nt = max((ch["nt"] for ch in chunks), default=1)
    tot_cols = max(meta["tot_cols"], 1)
    tot_idx16 = max(meta["tot_idx16"], 16)

    nc = bacc.Bacc("TRN2", target_bir_lowering=False, debug=False,
                   num_devices=NC, num_swdge_queues=NQUEUES)
    f32 = mybir.dt.float32
    ftt = nc.dram_tensor("ftt", [D_IN, npc], f32, kind="ExternalInput")
    w1t = nc.dram_tensor("w1t", [D_H, D_IN], f32, kind="ExternalInput")
    w2 = nc.dram_tensor("w2", [D_H, D_OUT], f32, kind="ExternalInput")
    idx_in = nc.dram_tensor("idx", [TILE, tot_idx16], mybir.dt.int16,
                            kind="ExternalInput")
    wq_in = nc.dram_tensor("wq", [TILE, tot_cols], f32, kind="ExternalInput")
    out_ext = nc.dram_tensor("out", [npc, D_OUT], f32, kind="ExternalOutput")

    x_full = nc.dram_tensor("x_full", [npad, D_OUT], f32, addr_space="Shared")
    bounce = nc.dram_tensor("bounce", [npc, D_OUT], f32)

    with tile.TileContext(nc) as tc:
        with (
            tc.tile_pool(name="persist", bufs=1) as persist,
            tc.tile_pool(name="msg", bufs=4) as msgp,
            tc.tile_pool(name="ft", bufs=2) as ftp,
            tc.tile_pool(name="psum", bufs=2, space="PSUM") as psump,
        ):
            idx_t = persist.tile([TILE, tot_idx16], mybir.dt.int16)
            wq_t = persist.tile([TILE, tot_cols], f32)
            acc = persist.tile([TILE, T, D_OUT], f32)
            h01 = persist.tile([TILE, T, D_OUT], f32)
            wc0 = persist.tile([TILE, D_OUT], f32)
            wc1 = persist.tile([TILE, D_OUT], f32)
            w1t_t = persist.tile([TILE, D_IN], f32)
            w2_t = persist.tile([TILE, D_OUT], f32)

            nc.sync.dma_start(idx_t[:], idx_in[:, :])
            nc.sync.dma_start(wq_t[:], wq_in[:, :])
            nc.sync.dma_start(w1t_t[:], w1t[:, :])
            nc.sync.dma_start(w2_t[:], w2[:, :])
            nc.gpsimd.load_library(mlp)

            # Wc = W1 @ W2 -> two [128, 64] tiles
            pwc = psump.tile([TILE, D_OUT], f32, tag="pw")
            nc.tensor.matmul(pwc[:], lhsT=w1t_t[:, 0:TILE], rhs=w2_t[:],
                             start=True, stop=True)
            nc.vector.tensor_copy(wc0[:], pwc[:])
            pwc2 = psump.tile([TILE, D_OUT], f32, tag="pw")
            nc.tensor.matmul(pwc2[:], lhsT=w1t_t[:, TILE:2 * TILE], rhs=w2_t[:],
                             start=True, stop=True)
            nc.vector.tensor_copy(wc1[:], pwc2[:])

            # G = F @ Wc (this core's shard) -> acc
            for t in range(T):
                ft_t = ftp.tile([TILE, 2, TILE], f32, tag="ft")
                nc.sync.dma_start(
                    ft_t[:],
                    ftt.ap()[:, t * TILE:(t + 1) * TILE]
                    .rearrange("(h k) m -> k h m", h=2))
                pg = psump.tile([TILE, D_OUT], f32, tag="pg")
                nc.tensor.matmul(pg[:], lhsT=ft_t[:, 0, :], rhs=wc0[:],
                                 start=True, stop=False)
                nc.tensor.matmul(pg[:], lhsT=ft_t[:, 1, :], rhs=wc1[:],
                                 start=False, stop=True)
                nc.vector.tensor_copy(acc[:, t, :], pg[:])

            def exchange():
                if BARRIERS:
                    tc.strict_bb_all_engine_barrier()
                if not EXCHANGE:
                    # debug mode: single-core semantics, copy own shard into
                    # our slice of x_full
                    nc.sync.dma_start(
                        bounce.ap().rearrange("(p t) d -> p (t d)", p=TILE),
                        acc[:].rearrange("p t d -> p (t d)"))
                    nc.sync.dma_start(x_full.ap()[0:npc, :], bounce.ap())
                    return
                nc.sync.dma_start(
                    bounce.ap().rearrange("(p t) d -> p (t d)", p=TILE),
                    acc[:].rearrange("p t d -> p (t d)"))
                cc = nc.gpsimd.collective_compute(
                    "AllGather", mybir.AluOpType.bypass,
                    replica_groups=[list(range(NC))],
                    ins=[bounce.ap().opt()],
                    outs=[x_full.ap().opt()],
                )
                last_cc[0] = cc
                if BARRIERS:
                    tc.strict_bb_all_engine_barrier()

            last_cc = [None]
            exchange()

            qrot = [0]

            def step(scale, add_h):
                call_i = 0
                for ci, ch in enumerate(chunks):
                    cols_tot = ch["cols_a"] + ch["cols_b"]
                    if cols_tot == 0:
                        for i, (a, b) in enumerate(ch["widths"]):
                            pass
                        nc.vector.memset(
                            acc[:, ch["t0"]:ch["t0"] + ch["nt"], :]
                            .rearrange("p t d -> p (t d)"), 0.0)
                        continue
                    m = msgp.tile([TILE, CHUNK_COLS, D_OUT], f32, tag="m")
                    mcol = 0
                    while (call_i < len(calls) and calls[call_i][4] == ci):
                        region, ioff, coff, cn, cid, _c0 = calls[call_i]
                        base = 0 if region == "A" else b_base
                        nidx = cn * TILE
                        g = nc.gpsimd.dma_gather(
                            m[:, mcol:mcol + cn, :],
                            x_full.ap()[base:base + min(REGION, npad), :],
                            idx_t[:, ioff:ioff + cn * 8],
                            nidx, nidx, D_OUT,
                            single_packet=False,
                            queue_num=qrot[0] % NQUEUES)
                        if last_cc[0] is not None:
                            add_dep_helper(g.ins, last_cc[0].ins, sync=True,
                                           reason="gather RAW on AllGather")
                        qrot[0] += 1
                        nc.vector.scalar_tensor_tensor(
                            out=m[:, mcol:mcol + cn, :],
                            in0=m[:, mcol:mcol + cn, :],
                            scalar=float(scale),
                            in1=wq_t[:, coff:coff + cn]
                            .to_broadcast([TILE, cn, D_OUT]),
                            op0=mybir.AluOpType.mult,
                            op1=mybir.AluOpType.mult)
                        mcol += cn
                        call_i += 1
                    # per-run reductions (A and B parts of a tile are not
                    # adjacent in the buffer: columns are laid out
                    # [A(t0..), B(t0..)]; reduce A-part into acc, then add
                    # B-part on top with a second reduce into a temp? To
                    # keep one reduce per tile run, reduce A into acc and
                    # B into acc via tensor_tensor add of reduced temp.
                    for part, key in (("A", 0), ("B", 1)):
                        coff0 = 0 if part == "A" else ch["cols_a"]
                        widths = [wdt[key] for wdt in ch["widths"]]
                        coff = coff0
                        trun = ch["t0"]
                        wi = 0
                        while wi < len(widths):
                            wtot = widths[wi]
                            nt = 1
                            while (wi + nt < len(widths)
                                   and widths[wi + nt] == wtot):
                                nt += 1
                            if wtot > 0:
                                srcap = (m[:, coff:coff + nt * wtot, :]
                                         .rearrange("p (t j) f -> p t f j",
                                                    j=wtot))
                                if part == "A":
                                    nc.vector.tensor_reduce(
                                        acc[:, trun:trun + nt, :], srcap,
                                        axis=mybir.AxisListType.X,
                                        op=mybir.AluOpType.add)
                                else:
                                    tmp = msgp.tile([TILE, max_nt, D_OUT],
                                                    f32, tag="btmp")
                                    nc.vector.tensor_reduce(
                                        tmp[:, 0:nt, :], srcap,
                                        axis=mybir.AxisListType.X,
                                        op=mybir.AluOpType.add)
                                    nc.vector.tensor_add(
                                        acc[:, trun:trun + nt, :],
                                        acc[:, trun:trun + nt, :],
                                        tmp[:, 0:nt, :])
                            elif part == "A":
                                nc.vector.memset(
                                    acc[:, trun:trun + nt, :]
                                    .rearrange("p t d -> p (t d)"), 0.0)
                            coff += nt * wtot
                            trun += nt
                            wi += nt
                if add_h:
                    nc.vector.tensor_add(
                        acc[:].rearrange("p t d -> p (t d)"),
                        acc[:].rearrange("p t d -> p (t d)"),
                        h01[:].rearrange("p t d -> p (t d)"))

            # step 1: P1 = prop(G)
            step(1.0, False)
            exchange()
            # step 2: x2 = prop(P1); h01 = 0.1 x2
            step(1.0, False)
            nc.scalar.mul(h01[:].rearrange("p t d -> p (t d)"),
                          acc[:].rearrange("p t d -> p (t d)"), ALPHA)
            exchange()
            # steps 3..12
            nk = K_STEPS if N_STEPS is None else max(0, N_STEPS - 2)
            for k in range(nk):
                step(1.0 - ALPHA, True)
                if k < nk - 1:
                    exchange()
            nc.sync.dma_start(
                out_ext.ap().rearrange("(p t) d -> p (t d)", p=TILE),
                acc[:].rearrange("p t d -> p (t d)"))

    nc.compile()
    return nc


# ---------------------------------------------------------------- entry

def kernel(features, edge_index, edge_weight, W1, b1, W2, b2):
    global LAST_EXEC_NS
    features = np.ascontiguousarray(np.asarray(features, dtype=np.float32))
    W1 = np.asarray(W1, dtype=np.float32)
    W2 = np.asarray(W2, dtype=np.float32)

    cores, meta = _prep(features, edge_index, edge_weight)
    nc = _build(meta)

    phys = meta["phys"]
    n, npc, npad = meta["n"], meta["npc"], meta["npad"]
    ftt_full = np.zeros((D_IN, npad), dtype=np.float32)
    ftt_full[:, meta["ftt_phys"]] = features.T
    in_maps = []
    for c in range(NC):
        idx = cores[c]["idx"]
        wq = cores[c]["w"]
        if meta["tot_idx16"] == 0:
            idx = np.zeros((TILE, 16), np.int16)
        if meta["tot_cols"] == 0:
            wq = np.zeros((TILE, 1), np.float32)
        in_maps.append({
            "ftt": np.ascontiguousarray(ftt_full[:, c * npc:(c + 1) * npc]),
            "w1t": np.ascontiguousarray(W1.T),
            "w2": np.ascontiguousarray(W2),
            "idx": idx,
            "wq": wq,
        })

    if TRACE:
        _install_ntff_hook()
    import time as _time
    global LAST_RUN_WALL_S
    _t0 = _time.time()
    res = run_bass_kernel_spmd(nc, in_maps, core_ids=list(range(NC)),
                               trace=TRACE)
    LAST_RUN_WALL_S = _time.time() - _t0
    LAST_EXEC_NS = res.exec_time_ns

    out_full = np.concatenate([res.results[c]["out"] for c in range(NC)], 0)
    out = out_full[phys[np.arange(n)]]
    return (np.ascontiguousarray(out, dtype=np.float32), 10)


# revision 17
# speedup vs baseline: 1.0766x; 1.0152x over previous
"""APPNP GNN kernel for 8 Trainium2 NeuronCores (Bass/Tile).

Math: with prop(x) = segment_sum(w * x[src], dst) and zero biases,
  x1 = prop(F @ W1); x2 = prop(x1 @ W2); h = x2
  x_{k+1} = 0.9 prop(x_k) + 0.1 h   (10 steps)
prop(X) @ W == prop(X @ W), so fold Wc = W1 @ W2, G = F @ Wc, and all 12
propagations run at D=64:
  P1 = prop(G); x2 = prop(P1) = h; then 10 APPNP steps.

Device mapping (per core): nodes are in-degree-sorted into 128-row tiles;
tiles are round-robined over the 8 cores. Each dst tile has an ELL slot
block [128 pos x W cols] (W = unified max in-degree of the tile across
cores). Messages are fetched from a replicated x buffer in DRAM with
dma_gather (int16 indices; the x buffer is covered by two overlapping
32768-row windows, each edge assigned to one window with per-node
balancing). DVE fuses (msgs * scale * w) and a strided tensor_reduce per
tile writes the new shard; an ncfw AllGather rebuilds x on every core
each step.
"""
import sys
import types

import numpy as np

sys.path.insert(0, "/opt/trn_rl_repo")

import concourse.bass as bass  # noqa: E402,F401
import concourse.tile as tile  # noqa: E402
from concourse import bacc, mybir  # noqa: E402
from concourse.bass_utils import run_bass_kernel_spmd  # noqa: E402
from concourse.library_config import mlp  # noqa: E402
from concourse.tile_rust import add_dep_helper  # noqa: E402

NC = 8
TILE = 128
D_IN, D_H, D_OUT = 256, 128, 64
K_STEPS = 10
ALPHA = 0.1
CHUNK_COLS = 96          # max ELL columns buffered per chunk
GATHER_COLS = 32         # 4096 idxs per dma_gather call (better 2-queue interleave)
REGION = 32768           # int16-addressable window rows

TRACE = False
LAST_EXEC_NS = None
EXCHANGE = True          # debug: disable collectives
N_STEPS = None           # debug: override number of propagation steps
NQUEUES = 2              # 2 SWDGE queues validated exact; 4 corrupts (see memory)
BARRIERS = False         # all-engine barriers around exchanges (not needed)
LAST_RUN_WALL_S = None


def _install_ntff_hook():
    import antenv
    if "antenv.axon_hooks" in sys.modules:
        return
    mod = types.ModuleType("antenv.axon_hooks")
    state = {"hook": None}
    mod.set_axon_ntff_profile_hook = lambda h: state.__setitem__("hook", h)
    mod.get_axon_ntff_profile_hook = lambda: state["hook"]
    sys.modules["antenv.axon_hooks"] = mod
    antenv.axon_hooks = mod
    from trn_agent_boot.trn_boot import _ntff_profile_via_ctypes
    mod.set_axon_ntff_profile_hook(
        _ntff_profile_via_ctypes("/opt/axon/libaxon_pjrt.so"))


# ---------------------------------------------------------------- host prep

def _prep(features, edge_index, edge_weight):
    n = features.shape[0]
    src = np.asarray(edge_index[0], dtype=np.int64)
    dst = np.asarray(edge_index[1], dtype=np.int64)
    w = np.asarray(edge_weight, dtype=np.float32)

    g_tiles = -(-n // TILE)
    g_tiles += (-g_tiles) % NC
    T = g_tiles // NC
    npc = T * TILE
    npad = npc * NC

    indeg = np.bincount(dst, minlength=n)
    order = np.argsort(-indeg, kind="stable")
    rank_of = np.empty(n, dtype=np.int64)
    rank_of[order] = np.arange(n)

    g = rank_of // TILE
    pos = rank_of % TILE
    core = g % NC
    t_in_core = g // NC
    phys = core * npc + pos * T + t_in_core
    # feature upload ordering: tile-major so each tile's 128 nodes are a
    # contiguous [256, 128] slice (partition p = matmul output partition)
    ftt_phys = core * npc + t_in_core * TILE + pos

    if npad <= REGION:
        b_base, two_regions = 0, False
    else:
        assert npad - REGION <= REGION
        b_base, two_regions = npad - REGION, True

    sp = phys[src]
    dp = phys[dst]
    dcore = dp // npc

    # per-core raw slot lists: slots[c][t*TILE+p] = ([(idx, w)...]A, [...]B)
    slot_a = [[[] for _ in range(T * TILE)] for _ in range(NC)]
    slot_b = [[[] for _ in range(T * TILE)] for _ in range(NC)]
    so = np.lexsort((sp, dp))
    sp_s, dp_s, w_s = sp[so], dp[so], w[so]
    bnd = np.flatnonzero(np.diff(dp_s)) + 1
    starts = np.concatenate([[0], bnd, [len(dp_s)]])
    for gi in range(len(starts) - 1):
        s0, s1 = starts[gi], starts[gi + 1]
        d = dp_s[s0]
        c = d // npc
        dl = d - c * npc
        p, t = dl // T, dl % T
        la = slot_a[c][t * TILE + p]
        lb = slot_b[c][t * TILE + p]
        if not two_regions:
            for j in range(s0, s1):
                la.append((int(sp_s[j]), float(w_s[j])))
        else:
            flex = []
            for j in range(s0, s1):
                s = int(sp_s[j])
                if s < b_base:
                    la.append((s, float(w_s[j])))
                elif s >= REGION:
                    lb.append((s - b_base, float(w_s[j])))
                else:
                    flex.append((s, float(w_s[j])))
            for s, wv in flex:
                if len(la) <= len(lb):
                    la.append((s, wv))
                else:
                    lb.append((s - b_base, wv))

    # unified widths per tile
    wa_u = np.zeros(T, dtype=np.int64)
    wb_u = np.zeros(T, dtype=np.int64)
    for c in range(NC):
        for t in range(T):
            base = t * TILE
            wa_u[t] = max(wa_u[t],
                          max((len(slot_a[c][base + p]) for p in range(TILE)),
                              default=0))
            wb_u[t] = max(wb_u[t],
                          max((len(slot_b[c][base + p]) for p in range(TILE)),
                              default=0))

    # chunks over tiles (shared structure)
    chunks = []
    t0 = 0
    while t0 < T:
        nt, cols = 0, 0
        while (t0 + nt < T
               and cols + wa_u[t0 + nt] + wb_u[t0 + nt] <= CHUNK_COLS):
            cols += wa_u[t0 + nt] + wb_u[t0 + nt]
            nt += 1
        assert nt > 0, f"tile {t0}: width {wa_u[t0]}+{wb_u[t0]} > {CHUNK_COLS}"
        chunks.append((t0, nt))
        t0 = t0 + nt

    # shared call list + per-core arrays
    calls = []           # (region, idx_off16, col_off, cn, chunk_id)
    chunk_descs = []
    idx_off = 0
    col_off = 0
    for cid, (t0, nt) in enumerate(chunks):
        cols_a = int(wa_u[t0:t0 + nt].sum())
        cols_b = int(wb_u[t0:t0 + nt].sum())
        for region, colsn in (("A", cols_a), ("B", cols_b)):
            c0 = 0
            while c0 < colsn:
                cn = min(GATHER_COLS, colsn - c0)
                calls.append((region, idx_off, col_off, cn, cid, c0))
                idx_off += cn * 8
                col_off += cn
                c0 += cn
        chunk_descs.append({
            "t0": t0, "nt": nt, "cols_a": cols_a, "cols_b": cols_b,
            "widths": [(int(wa_u[t]), int(wb_u[t]))
                       for t in range(t0, t0 + nt)]})
    tot_cols = col_off
    tot_idx16 = idx_off

    cores = []
    for c in range(NC):
        idx_arr = np.zeros((16, tot_idx16), dtype=np.int16)
        w_arr = np.zeros((TILE, tot_cols), dtype=np.float32)
        for (region, ioff, coff, cn, cid, c0) in calls:
            ch = chunk_descs[cid]
            t0, nt = ch["t0"], ch["nt"]
            widths = wa_u if region == "A" else wb_u
            slots = slot_a[c] if region == "A" else slot_b[c]
            # region columns of the chunk, flattened (t, j); call covers
            # [c0, c0+cn)
            col_list = []
            for t in range(t0, t0 + nt):
                for j in range(int(widths[t])):
                    col_list.append((t, j))
            blk_i = np.zeros((cn, TILE), dtype=np.int16)
            blk_w = np.zeros((cn, TILE), dtype=np.float32)
            for k in range(cn):
                t, j = col_list[c0 + k]
                base = t * TILE
                for p in range(TILE):
                    sl = slots[base + p]
                    if j < len(sl):
                        blk_i[k, p] = sl[j][0]
                        blk_w[k, p] = sl[j][1]
            idx_arr[:, ioff:ioff + cn * 8] = blk_i.reshape(-1, 16).T
            w_arr[:, coff:coff + cn] = blk_w.T
        cores.append({
            "idx": np.ascontiguousarray(np.tile(idx_arr, (8, 1))),
            "w": np.ascontiguousarray(w_arr)})

    meta = {"n": n, "T": T, "npc": npc, "npad": npad, "b_base": b_base,
            "phys": phys, "ftt_phys": ftt_phys, "chunks": chunk_descs,
            "calls": calls, "tot_cols": tot_cols, "tot_idx16": tot_idx16}
    return cores, meta


# ---------------------------------------------------------------- builder

def _build(meta):
    T = meta["T"]
    npc = meta["npc"]
    npad = meta["npad"]
    b_base = meta["b_base"]
    chunks = meta["chunks"]
    calls = meta["calls"]
    max_nt = max((ch["nt"] for ch in chunks), default=1)
    tot_cols = max(meta["tot_cols"], 1)
    tot_idx16 = max(meta["tot_idx16"], 16)

    nc = bacc.Bacc("TRN2", target_bir_lowering=False, debug=False,
                   num_devices=NC, num_swdge_queues=NQUEUES)
    f32 = mybir.dt.float32
    ftt = nc.dram_tensor("ftt", [D_IN, npc], f32, kind="ExternalInput")
    w1t = nc.dram_tensor("w1t", [D_H, D_IN], f32, kind="ExternalInput")
    w2 = nc.dram_tensor("w2", [D_H, D_OUT], f32, kind="ExternalInput")
    idx_in = nc.dram_tensor("idx", [TILE, tot_idx16], mybir.dt.int16,
                            kind="ExternalInput")
    wq_in = nc.dram_tensor("wq", [TILE, tot_cols], f32, kind="ExternalInput")
    out_ext = nc.dram_tensor("out", [npc, D_OUT], f32, kind="ExternalOutput")

    x_full = nc.dram_tensor("x_full", [npad, D_OUT], f32, addr_space="Shared")
    bounce = nc.dram_tensor("bounce", [npc, D_OUT], f32)

    with tile.TileContext(nc) as tc:
        with (
            tc.tile_pool(name="persist", bufs=1) as persist,
            tc.tile_pool(name="msg", bufs=4) as msgp,
            tc.tile_pool(name="ft", bufs=2) as ftp,
            tc.tile_pool(name="psum", bufs=2, space="PSUM") as psump,
        ):
            idx_t = persist.tile([TILE, tot_idx16], mybir.dt.int16)
            wq_t = persist.tile([TILE, tot_cols], f32)
            acc = persist.tile([TILE, T, D_OUT], f32)
            h01 = persist.tile([TILE, T, D_OUT], f32)
            wc0 = persist.tile([TILE, D_OUT], f32)
            wc1 = persist.tile([TILE, D_OUT], f32)
            w1t_t = persist.tile([TILE, D_IN], f32)
            w2_t = persist.tile([TILE, D_OUT], f32)

            nc.sync.dma_start(idx_t[:], idx_in[:, :])
            nc.sync.dma_start(wq_t[:], wq_in[:, :])
            nc.sync.dma_start(w1t_t[:], w1t[:, :])
            nc.sync.dma_start(w2_t[:], w2[:, :])
            nc.gpsimd.load_library(mlp)

            # Wc = W1 @ W2 -> two [128, 64] tiles
            pwc = psump.tile([TILE, D_OUT], f32, tag="pw")
            nc.tensor.matmul(pwc[:], lhsT=w1t_t[:, 0:TILE], rhs=w2_t[:],
                             start=True, stop=True)
            nc.vector.tensor_copy(wc0[:], pwc[:])
            pwc2 = psump.tile([TILE, D_OUT], f32, tag="pw")
            nc.tensor.matmul(pwc2[:], lhsT=w1t_t[:, TILE:2 * TILE], rhs=w2_t[:],
                             start=True, stop=True)
            nc.vector.tensor_copy(wc1[:], pwc2[:])

            # G = F @ Wc (this core's shard) -> acc
            for t in range(T):
                ft_t = ftp.tile([TILE, 2, TILE], f32, tag="ft")
                nc.sync.dma_start(
                    ft_t[:],
                    ftt.ap()[:, t * TILE:(t + 1) * TILE]
                    .rearrange("(h k) m -> k h m", h=2))
                pg = psump.tile([TILE, D_OUT], f32, tag="pg")
                nc.tensor.matmul(pg[:], lhsT=ft_t[:, 0, :], rhs=wc0[:],
                                 start=True, stop=False)
                nc.tensor.matmul(pg[:], lhsT=ft_t[:, 1, :], rhs=wc1[:],
                                 start=False, stop=True)
                nc.vector.tensor_copy(acc[:, t, :], pg[:])

            def exchange():
                if BARRIERS:
                    tc.strict_bb_all_engine_barrier()
                if not EXCHANGE:
                    # debug mode: single-core semantics, copy own shard into
                    # our slice of x_full
                    nc.sync.dma_start(
                        bounce.ap().rearrange("(p t) d -> p (t d)", p=TILE),
                        acc[:].rearrange("p t d -> p (t d)"))
                    nc.sync.dma_start(x_full.ap()[0:npc, :], bounce.ap())
                    return
                nc.sync.dma_start(
                    bounce.ap().rearrange("(p t) d -> p (t d)", p=TILE),
                    acc[:].rearrange("p t d -> p (t d)"))
                cc = nc.gpsimd.collective_compute(
                    "AllGather", mybir.AluOpType.bypass,
                    replica_groups=[list(range(NC))],
                    ins=[bounce.ap().opt()],
                    outs=[x_full.ap().opt()],
                )
                last_cc[0] = cc
                if BARRIERS:
                    tc.strict_bb_all_engine_barrier()

            last_cc = [None]
            exchange()

            qrot = [0]

            def step(scale, add_h):
                call_i = 0
                for ci, ch in enumerate(chunks):
                    cols_tot = ch["cols_a"] + ch["cols_b"]
                    if cols_tot == 0:
                        for i, (a, b) in enumerate(ch["widths"]):
                            pass
                        nc.vector.memset(
                            acc[:, ch["t0"]:ch["t0"] + ch["nt"], :]
                            .rearrange("p t d -> p (t d)"), 0.0)
                        continue
                    m = msgp.tile([TILE, CHUNK_COLS, D_OUT], f32, tag="m")
                    mcol = 0
                    while (call_i < len(calls) and calls[call_i][4] == ci):
                        region, ioff, coff, cn, cid, _c0 = calls[call_i]
                        base = 0 if region == "A" else b_base
                        nidx = cn * TILE
                        g = nc.gpsimd.dma_gather(
                            m[:, mcol:mcol + cn, :],
                            x_full.ap()[base:base + min(REGION, npad), :],
                            idx_t[:, ioff:ioff + cn * 8],
                            nidx, nidx, D_OUT,
                            single_packet=False,
                            queue_num=qrot[0] % NQUEUES)
                        if last_cc[0] is not None:
                            add_dep_helper(g.ins, last_cc[0].ins, sync=True,
                                           reason="gather RAW on AllGather")
                        qrot[0] += 1
                        nc.vector.scalar_tensor_tensor(
                            out=m[:, mcol:mcol + cn, :],
                            in0=m[:, mcol:mcol + cn, :],
                            scalar=float(scale),
                            in1=wq_t[:, coff:coff + cn]
                            .to_broadcast([TILE, cn, D_OUT]),
                            op0=mybir.AluOpType.mult,
                            op1=mybir.AluOpType.mult)
                        mcol += cn
                        call_i += 1
                    # per-run reductions (A and B parts of a tile are not
                    # adjacent in the buffer: columns are laid out
                    # [A(t0..), B(t0..)]; reduce A-part into acc, then add
                    # B-part on top with a second reduce into a temp? To
                    # keep one reduce per tile run, reduce A into acc and
                    # B into acc via tensor_tensor add of reduced temp.
                    for part, key in (("A", 0), ("B", 1)):
                        coff0 = 0 if part == "A" else ch["cols_a"]
                        widths = [wdt[key] for wdt in ch["widths"]]
                        coff = coff0
                        trun = ch["t0"]
                        wi = 0
                        while wi < len(widths):
                            wtot = widths[wi]
                            nt = 1
                            while (wi + nt < len(widths)
                                   and widths[wi + nt] == wtot):
                                nt += 1
                            if wtot > 0:
                                srcap = (m[:, coff:coff + nt * wtot, :]
                                         .rearrange("p (t j) f -> p t f j",
                                                    j=wtot))
                                if part == "A":
                                    nc.vector.tensor_reduce(
                                        acc[:, trun:trun + nt, :], srcap,
                                        axis=mybir.AxisListType.X,
                                        op=mybir.AluOpType.add)
                                else:
                                    tmp = msgp.tile([TILE, max_nt, D_OUT],
                                                    f32, tag="btmp")
                                    nc.vector.tensor_reduce(
                                        tmp[:, 0:nt, :], srcap,
                                        axis=mybir.AxisListType.X,
                                        op=mybir.AluOpType.add)
                                    nc.vector.tensor_add(
                                        acc[:, trun:trun + nt, :],
                                        acc[:, trun:trun + nt, :],
                                        tmp[:, 0:nt, :])
                            elif part == "A":
                                nc.vector.memset(
                                    acc[:, trun:trun + nt, :]
                                    .rearrange("p t d -> p (t d)"), 0.0)
                            coff += nt * wtot
                            trun += nt
                            wi += nt
                if add_h:
                    nc.vector.tensor_add(
                        acc[:].rearrange("p t d -> p (t d)"),
                        acc[:].rearrange("p t d -> p (t d)"),
                        h01[:].rearrange("p t d -> p (t d)"))

            # step 1: P1 = prop(G)
            step(1.0, False)
            exchange()
            # step 2: x2 = prop(P1); h01 = 0.1 x2
            step(1.0, False)
            nc.scalar.mul(h01[:].rearrange("p t d -> p (t d)"),
                          acc[:].rearrange("p t d -> p (t d)"), ALPHA)
            exchange()
            # steps 3..12
            nk = K_STEPS if N_STEPS is None else max(0, N_STEPS - 2)
            for k in range(nk):
                step(1.0 - ALPHA, True)
                if k < nk - 1:
                    exchange()
            nc.sync.dma_start(
                out_ext.ap().rearrange("(p t) d -> p (t d)", p=TILE),
                acc[:].rearrange("p t d -> p (t d)"))

    nc.compile()
    return nc


# ---------------------------------------------------------------- entry

def kernel(features, edge_index, edge_weight, W1, b1, W2, b2):
    global LAST_EXEC_NS
    features = np.ascontiguousarray(np.asarray(features, dtype=np.float32))
    W1 = np.asarray(W1, dtype=np.float32)
    W2 = np.asarray(W2, dtype=np.float32)

    cores, meta = _prep(features, edge_index, edge_weight)
    nc = _build(meta)

    phys = meta["phys"]
    n, npc, npad = meta["n"], meta["npc"], meta["npad"]
    ftt_full = np.zeros((D_IN, npad), dtype=np.float32)
    ftt_full[:, meta["ftt_phys"]] = features.T
    in_maps = []
    for c in range(NC):
        idx = cores[c]["idx"]
        wq = cores[c]["w"]
        if meta["tot_idx16"] == 0:
            idx = np.zeros((TILE, 16), np.int16)
        if meta["tot_cols"] == 0:
            wq = np.zeros((TILE, 1), np.float32)
        in_maps.append({
            "ftt": np.ascontiguousarray(ftt_full[:, c * npc:(c + 1) * npc]),
            "w1t": np.ascontiguousarray(W1.T),
            "w2": np.ascontiguousarray(W2),
            "idx": idx,
            "wq": wq,
        })

    if TRACE:
        _install_ntff_hook()
    import time as _time
    global LAST_RUN_WALL_S
    _t0 = _time.time()
    res = run_bass_kernel_spmd(nc, in_maps, core_ids=list(range(NC)),
                               trace=TRACE)
    LAST_RUN_WALL_S = _time.time() - _t0
    LAST_EXEC_NS = res.exec_time_ns

    out_full = np.concatenate([res.results[c]["out"] for c in range(NC)], 0)
    out = out_full[phys[np.arange(n)]]
    return (np.ascontiguousarray(out, dtype=np.float32), 10)
